# revision 27
# baseline (speedup 1.0000x reference)
"""CCA few-shot scoring kernel for Trainium2 (8 NeuronCores, SPMD).

Inputs (full): spt [1,5,3840,5,5] f32, qry [75,3840,5,5] f32.
Output: sim [75,5] f32.

Sharding: data-parallel over queries. 75 queries padded to 80; each of the
8 cores handles 10 queries against the full replicated support set.

v2 design (vs v0 baseline):
  - no elementwise centering: the channel-mean subtraction is folded into
    the PE chains as rank-2 correction matmuls; per-level column sums ride
    the corr/gramS chains for free via a ones-column in the stationary
  - gramQ computed directly in compressed block-diagonal form (10 per-chunk
    matmuls at the same total column cost)
  - Activation engine strictly minimal: 2 softmax EXPs + batched rsqrts
    (column-form ns2/nq2 share one LN/EXP pair); table pre-warmed at t=0
  - no PSUM->SBUF staging copies for broadcast fields: tensor ops read
    PSUM operands directly; per-way/per-query gram quadratic forms use
    sliced matmuls instead of mask multiplies
  - dot = sum_t (BIND^T(corr . As))*aq - avoids the attn_q broadcast
  - exact GN_EPS handling (per-partition bias APs), removing the ~2e-3
    systematic error of the eps-folding approximation
  - DMA split across 4 queues (SP/Act HWDGE, DVE HWDGE, Pool SWDGE)
"""

import json

import numpy as np
from concourse import bass, mybir
from concourse import bass2jax as _b2j
from concourse import bass_utils as _bu
from concourse.tile import TileContext
from concourse.bass_utils import run_bass_kernel_spmd


def _split_multiwaits(bir_json: bytes) -> bytes:
    """Walrus in this env allows one sync-wait per compute instruction.

    Split every multi-wait instruction: hoist all but the last wait onto
    fresh single-wait RegisterMove carriers (same engine, immediately
    preceding), cloned from the preamble zero-reg writes.
    """
    j = json.loads(bir_json)
    tmpl = {}
    for f in j["functions"]:
        for b in f["blocks"]:
            for i in b["instructions"]:
                if i["opcode"] == "RegisterMove":
                    for o in i.get("outs", []):
                        if str(o.get("regref", "")).endswith("_zero"):
                            tmpl.setdefault(i["engine"], i)
    uid = [0]

    def carrier(engine, wait):
        t = tmpl[engine]
        uid[0] += 1
        return {
            "debug": t.get("debug", 0),
            "engine": engine,
            "ins": [dict(x) for x in t["ins"]],
            "name": f"IW-{uid[0]}",
            "opcode": "RegisterMove",
            "outs": [dict(x) for x in t["outs"]],
            "sync_info": {"on_update": [], "on_wait": [wait]},
        }

    for f in j["functions"]:
        for b in f["blocks"]:
            out = []
            for i in b["instructions"]:
                si = i.get("sync_info")
                ow = si.get("on_wait") if si else None
                if ow and len(ow) > 1:
                    for w in ow[:-1]:
                        out.append(carrier(i["engine"], w))
                    si["on_wait"] = [ow[-1]]
                out.append(i)
            b["instructions"] = out
    return json.dumps(j).encode()


_orig_cbk = _bu.compile_bir_kernel


def _patched_cbk(bir_json, tmpdir, neff_name="file.neff"):
    return _orig_cbk(_split_multiwaits(bir_json), tmpdir, neff_name=neff_name)


for _mod in (_b2j, _bu):
    if getattr(_mod, "compile_bir_kernel", None) is _orig_cbk:
        _mod.compile_bir_kernel = _patched_cbk

F32 = mybir.dt.float32
BF16 = mybir.dt.bfloat16
AX = mybir.AxisListType.X
ADD = mybir.AluOpType.add
SUB = mybir.AluOpType.subtract
MUL = mybir.AluOpType.mult
EXP = mybir.ActivationFunctionType.Exp
LN = mybir.ActivationFunctionType.Ln
COPY = mybir.ActivationFunctionType.Copy
SQUARE = mybir.ActivationFunctionType.Square

HYPER = [256, 512, 1024, 2048]
C = 3840
WAY = 5
SS = 25          # fs*fs
NQ = 75
NQL = 10         # queries per core
NCORES = 8
P = 128
WS = WAY * SS    # 125
QT = NQL * SS    # 250
T_ATTN = 5.0
L2_EPS = 1e-6
GN_EPS = 1e-5
NLV = 4
LCH = [d // P for d in HYPER]     # chunks per level: 2,4,8,16
SW = 125                          # s chunk width

# bf16 const image columns
C_I = 0        # I125 [125,125]
C_BI = 125     # BIND [125,5]
C_B5 = 130     # BIND5 = BIND*0.2 [125,5]
C_BIT = 135    # BINDT [5,125]
C_WM = 260     # WMASK [125,125]
C_OR = 385     # ONESR [1,128]
C_OC = 513     # ONESC [128,1]
C_DV = 514     # dvec [1,4]
NCB = 518
# f32 const image: IF5
NCF = 5

_CACHE = {}
LINEARIZE = False

# emit order of level chains (matched to DMA arrival)
CHAIN_ORDER = [0, 3, 2, 1]


def _build_nc():
    nc = bass.Bass()
    bexp = 0.5 * float(np.log(SS - 1.0)) - float(np.log(T_ATTN))
    for val in (L2_EPS, bexp, float(np.log(5.0)), 0.02):
        t = nc.alloc_sbuf_tensor(f"const-f32-{val}", [128, 1], F32)
        nc.gpsimd.memset(t.ap(), val)
        nc.const_aps.aps[(F32, val)] = t.ap()
    warm = nc.alloc_sbuf_tensor("warm", [1, 1], F32)
    nc.gpsimd.memset(warm.ap(), 0.25)
    nc.all_engine_barrier()

    sd = [nc.declare_dram_parameter(f"s{l}", [P, LCH[l] * SW], BF16,
                                    isOutput=False) for l in range(NLV)]
    qd = [nc.declare_dram_parameter(f"q{l}", [P, LCH[l] * QT], BF16,
                                    isOutput=False) for l in range(NLV)]
    cb_d = nc.declare_dram_parameter("cb", [P, NCB], BF16, isOutput=False)
    cf_d = nc.declare_dram_parameter("cf", [P, NCF], F32, isOutput=False)
    out_d = nc.declare_dram_parameter("out", [WAY, NQL], F32, isOutput=True)

    with TileContext(nc, linearize=LINEARIZE) as tc:
        with (
            tc.tile_pool(name="const", bufs=1) as cpool,
            tc.tile_pool(name="data", bufs=1) as data,
            tc.tile_pool(name="meanp", bufs=1) as meanp,
            tc.tile_pool(name="big", bufs=4) as big,
            tc.tile_pool(name="small", bufs=4) as small,
            tc.tile_pool(name="ps_chain", bufs=1, space="PSUM") as ps_chain,
            tc.tile_pool(name="ps_work", bufs=1, space="PSUM") as ps_work,
            tc.tile_pool(name="ps_acc", bufs=1, space="PSUM") as ps_acc,
        ):
            # ---- data + const DMA loads, 4 queues ----
            st_ = [data.tile([P, LCH[l] * SW], BF16, tag=f"s{l}",
                             name=f"s{l}") for l in range(NLV)]
            qt_ = [data.tile([P, LCH[l] * QT], BF16, tag=f"q{l}",
                             name=f"q{l}") for l in range(NLV)]
            CB = cpool.tile([P, NCB], BF16)
            CF = cpool.tile([P, NCF], F32)

            # Pool (SWDGE): level 0
            nc.gpsimd.dma_start(out=st_[0][:, :], in_=sd[0][:, :])
            nc.gpsimd.dma_start(out=qt_[0][:, :], in_=qd[0][:, :])
            # SP (HWDGE): q3, s2, s1
            nc.sync.dma_start(out=qt_[3][:, :], in_=qd[3][:, :])
            nc.sync.dma_start(out=st_[2][:, :], in_=sd[2][:, :])
            nc.sync.dma_start(out=st_[1][:, :], in_=sd[1][:, :])
            # Act (HWDGE): s3, q2, q1, consts
            nc.scalar.dma_start(out=st_[3][:, :], in_=sd[3][:, :])
            nc.scalar.dma_start(out=qt_[2][:, :], in_=qd[2][:, :])
            nc.scalar.dma_start(out=qt_[1][:, :], in_=qd[1][:, :])
            nc.scalar.dma_start(out=CB[:, :], in_=cb_d[:, :])
            nc.scalar.dma_start(out=CF[:, :], in_=cf_d[:, :])

            # warm the ln/exp activation table during the DMA shadow
            warm_out = small.tile([1, 1], F32, tag="warm")
            nc.scalar.activation(warm_out[:, :], warm.ap(), LN)

            I125 = CB[0:WS, C_I:C_I + WS]
            BIND = CB[0:WS, C_BI:C_BI + WAY]
            BIND5 = CB[0:WS, C_B5:C_B5 + WAY]
            BINDT = CB[0:WAY, C_BIT:C_BIT + WS]
            WMASK = CB[0:WS, C_WM:C_WM + WS]
            ONESR125 = CB[0:1, C_OR:C_OR + WS]
            ONESC = CB[0:P, C_OC:C_OC + 1]
            DVEC = CB[0:1, C_DV:C_DV + 4]
            IF5 = CF[0:WAY, 0:WAY]

            # ---- chains ----
            # chA bank [128, 512] per level:
            #   [0:125, 0:250]   corr
            #   [0:125, 250:375] gramS
            #   [0:1, 375:500]   csS row (level channel-sums of s)
            # chB banks (2, rotating):
            #   [0:125, 0:250]   gramQ full halves
            #   [0:1, 250:500]   csQ row
            chA = [ps_chain.tile([P, 512], F32, tag=f"chA{l}",
                                 name=f"chA{l}") for l in range(NLV)]
            rs16 = [meanp.tile([1, 375], BF16, name=f"rs16_{l}")
                    for l in range(NLV)]
            gqraw = [None] * NLV

            for li, l in enumerate(CHAIN_ORDER):
                m = LCH[l]
                ch = chA[l]
                chB = ps_chain.tile([P, 512], F32, tag=f"chB{li % 2}",
                                    name=f"chB{l}")
                mb = ps_acc.tile([1, 375], F32, tag="accb",
                                 name=f"mb{l}")
                for k in range(m):
                    sk = st_[l][:, k * SW:k * SW + WS]         # [128,125]
                    qk = qt_[l][:, k * QT:(k + 1) * QT]        # [128,250]
                    nc.tensor.matmul(ch[0:WS, 0:QT], sk, qk,
                                     start=(k == 0), stop=False)
                    nc.tensor.matmul(ch[0:WS, QT:QT + WS], sk, sk,
                                     start=False, stop=False)
                    nc.tensor.matmul(chB[0:WS, 0:WS], qk[:, 0:WS],
                                     qk[:, 0:WS], start=(k == 0),
                                     stop=False)
                    nc.tensor.matmul(chB[0:WS, WS:QT], qk[:, WS:QT],
                                     qk[:, WS:QT], start=False,
                                     stop=(k == m - 1))
                    nc.tensor.matmul(mb[0:1, 0:250], ONESC, qk,
                                     start=(k == 0), stop=False)
                    nc.tensor.matmul(mb[0:1, 250:375], ONESC, sk,
                                     start=False, stop=(k == m - 1))
                # per-level raw sums -> bf16 row tile (cols 0:250 = csQ,
                # 250:375 = csS); raw gramQ out of PSUM (frees chB)
                nc.scalar.activation(rs16[l][0:1, 0:375], mb[0:1, 0:375],
                                     COPY)
                gq_r = big.tile([WS, QT], BF16, tag="gqraw",
                                name=f"gqraw{l}")
                nc.vector.tensor_copy(gq_r[:, :], chB[0:WS, 0:QT])
                gqraw[l] = gq_r

            # ---- global means + correction rows ----
            ps_mu = ps_work.tile([1, 375], F32, tag="pw", name="psmu")
            ONES1 = CB[0:1, C_OC:C_OC + 1]
            for l in range(NLV):
                nc.tensor.matmul(ps_mu[:, :], ONES1, rs16[l][:, :],
                                 start=(l == 0), stop=(l == NLV - 1))
            mup16 = meanp.tile([1, 375], BF16, name="mup16")
            nc.scalar.activation(mup16[:, :], ps_mu[:, :], COPY,
                                 scale=1.0 / C)
            mun16 = meanp.tile([1, 375], BF16, name="mun16")
            nc.scalar.activation(mun16[:, :], ps_mu[:, :], COPY,
                                 scale=-1.0 / C)
            a1_16 = [meanp.tile([1, 375], BF16, name=f"a1_16_{l}")
                     for l in range(NLV)]
            for l in range(NLV):
                ps_dl = ps_work.tile([1, 375], F32, tag="pw",
                                     name=f"psd{l}")
                nc.tensor.matmul(ps_dl[:, :], DVEC[0:1, l:l + 1],
                                 mup16[:, :], start=True, stop=True)
                nc.vector.tensor_tensor(a1_16[l][:, :], ps_dl[:, :],
                                        rs16[l][:, :], SUB)

            # correction matmuls (rank-2 per region); close chA groups
            for l in range(NLV):
                ch = chA[l]
                muS_n = mun16[0:1, 250:375]
                muS_p = mup16[0:1, 250:375]
                muQ_p = mup16[0:1, 0:250]
                a1S = a1_16[l][0:1, 250:375]
                nc.tensor.matmul(ch[0:WS, 0:QT], muS_n,
                                 rs16[l][0:1, 0:250],
                                 start=False, stop=False)
                nc.tensor.matmul(ch[0:WS, 0:QT], a1S, muQ_p,
                                 start=False, stop=False)
                nc.tensor.matmul(ch[0:WS, QT:QT + WS], muS_n,
                                 rs16[l][0:1, 250:375],
                                 start=False, stop=False)
                nc.tensor.matmul(ch[0:WS, QT:QT + WS], a1S, muS_p,
                                 start=False, stop=True)

            # gramQ corrections: rank-2 outer products into a work tile,
            # then one SBUF add onto the raw copy (per half: x,y in-half)
            gq16c = [None] * NLV
            for l in range(NLV):
                ps_gq = ps_chain.tile([WS, QT], F32, tag=f"chB{l % 2}",
                                      name=f"psgq{l}")
                for h in range(2):
                    sl = slice(h * WS, (h + 1) * WS)
                    nc.tensor.matmul(ps_gq[:, sl], mun16[0:1, sl],
                                     rs16[l][0:1, sl],
                                     start=(h == 0), stop=False)
                    nc.tensor.matmul(ps_gq[:, sl], a1_16[l][0:1, sl],
                                     mup16[0:1, sl],
                                     start=False, stop=(h == 1))
                gq = big.tile([WS, QT], BF16, tag="gq16", name=f"gq16c{l}")
                nc.vector.tensor_tensor(gq[:, :], ps_gq[:, :],
                                        gqraw[l][:, :], ADD)
                gq16c[l] = gq

            # ---- per-level post stages, software-pipelined (stage-major) ---
            V = [dict() for _ in range(NLV)]
            _rot = [0]
            _banks = [(ps_work, "pw"), (ps_chain, "chA0"),
                      (ps_chain, "chB0"), (ps_chain, "chA1"),
                      (ps_chain, "chB1"), (ps_chain, "chA2"),
                      (ps_chain, "chA3")]

            def pwork(shape, name, dtype=F32):
                pool, tag = _banks[_rot[0] % len(_banks)]
                _rot[0] += 1
                return pool.tile(shape, dtype, tag=tag,
                                 name=f"{name}{_rot[0]}")

            # acc bank [5,20]: cols 0:10 |s_pool|^2, 10:20 |q_pool|^2
            acc = ps_acc.tile([WAY, 20], F32, tag="accb", name="acc")
            dotacc = small.tile([WAY, 4 * NQL], F32, name="dotacc")

            def st0(l, v):
                """PSUM -> SBUF bf16 copies; frees chain bank."""
                ch = chA[l]
                c2 = big.tile([WS, QT], BF16, tag="c2", name=f"c2_{l}")
                nc.vector.tensor_copy(c2[:, :], ch[0:WS, 0:QT])
                v["c2"] = c2
                gs16 = big.tile([WS, WS], BF16, tag="gs16", name=f"gs{l}")
                nc.scalar.activation(gs16[:, :], ch[0:WS, QT:QT + WS], COPY)
                v["gs16"] = gs16
                v["gq16"] = gq16c[l]

            def st1(l, v):
                """diagonals -> inv-norm columns + eps bias columns."""
                md = big.tile([WS, WS], BF16, tag="md", name=f"md{l}")
                nc.gpsimd.tensor_tensor(md[:, :], v["gs16"][:, :], I125, MUL)
                mdq = big.tile([WS, QT], BF16, tag="mdq", name=f"mdq{l}")
                nc.gpsimd.tensor_tensor(mdq[:, 0:WS], v["gq16"][:, 0:WS],
                                        I125, MUL)
                nc.gpsimd.tensor_tensor(mdq[:, WS:QT], v["gq16"][:, WS:QT],
                                        I125, MUL)
                invcol = small.tile([WS, 3], F32, tag="invcol",
                                    name=f"invcol{l}")
                nc.vector.tensor_reduce(
                    invcol[:, 0:1],
                    md[:, :].rearrange("p (g t) -> p g t", t=WS), AX, ADD)
                nc.vector.tensor_reduce(
                    invcol[:, 1:3],
                    mdq[:, :].rearrange("p (g t) -> p g t", t=WS), AX, ADD)
                v["invcol"] = invcol
                bias3 = small.tile([WS, 3], F32, tag="bias3",
                                   name=f"bias3{l}")
                nc.vector.tensor_scalar_mul(bias3[:, :], invcol[:, :],
                                            (SS - 1.0) * GN_EPS)
                v["bias3"] = bias3

            def st2(l, v):
                """rsqrt of ns2|nq2 columns (shared LN/EXP)."""
                lncol = small.tile([WS, 3], F32, tag="lncol",
                                   name=f"lncol{l}")
                nc.scalar.activation(lncol[:, :], v["invcol"][:, :], LN,
                                     bias=L2_EPS)
                invnF = small.tile([WS, 3], F32, tag="invnF",
                                   name=f"invnF{l}")
                nc.scalar.activation(invnF[:, :], lncol[:, :], EXP,
                                     scale=-0.5)
                v["invnF"] = invnF
                invn16 = small.tile([WS, 3], BF16, tag="invn16",
                                    name=f"invn16{l}")
                nc.gpsimd.tensor_copy(invn16[:, :], invnF[:, :])
                v["invn16"] = invn16

            def st3(l, v):
                """t-scale: y = c2 * invnq (via transpose + bcast); g."""
                invn16 = v["invn16"]
                ps_nqrow = pwork([1, QT], "nqrow")
                nc.tensor.matmul(ps_nqrow[0:1, 0:WS], invn16[:, 1:2], I125,
                                 start=True, stop=False)
                nc.tensor.matmul(ps_nqrow[0:1, WS:QT], invn16[:, 2:3], I125,
                                 start=False, stop=True)
                nqrow16 = small.tile([1, QT], BF16, tag="nqrow16",
                                     name=f"nqrow{l}")
                nc.scalar.activation(nqrow16[:, :], ps_nqrow[:, :], COPY)
                ps_nqb = pwork([WS, QT], "nqb")
                nc.tensor.matmul(ps_nqb[:, :], ONESR125, nqrow16[:, :],
                                 start=True, stop=True)
                y = big.tile([WS, QT], BF16, tag="y", name=f"y{l}")
                nc.vector.tensor_tensor(y[:, :], v["c2"][:, :],
                                        ps_nqb[:, :], MUL)
                v["y"] = y
                g = big.tile([WS, QT], BF16, tag="g", name=f"g{l}")
                nc.vector.tensor_scalar_mul(g[:, :], v["c2"][:, :],
                                            v["invnF"][:, 0:1])
                v["g"] = g

            def st4(l, v):
                """t-stats and s-stats."""
                y, g = v["y"], v["g"]
                vall = small.tile([WS, 20], F32, tag="vall", name=f"vall{l}")
                v["vall"] = vall
                s1 = small.tile([WS, NQL], F32, tag="s1", name=f"s1_{l}")
                nc.vector.tensor_reduce(
                    s1[:, :], y[:, :].rearrange("p (g t) -> p g t", t=SS),
                    AX, ADD)
                ysq = big.tile([WS, QT], BF16, tag="ysq", name=f"ysq{l}")
                nc.gpsimd.tensor_tensor(ysq[:, :], y[:, :], y[:, :], MUL)
                s2 = small.tile([WS, NQL], F32, tag="s2", name=f"s2_{l}")
                nc.vector.tensor_reduce(
                    s2[:, :], ysq[:, :].rearrange("p (g t) -> p g t", t=SS),
                    AX, ADD)
                s1p = small.tile([WS, NQL], F32, tag="s1p", name=f"s1p{l}")
                nc.gpsimd.tensor_scalar_mul(s1p[:, :], s1[:, :], 0.2)
                s1sq = small.tile([WS, NQL], F32, tag="s1sq",
                                  name=f"s1sq{l}")
                nc.gpsimd.tensor_tensor(s1sq[:, :], s1p[:, :], s1p[:, :],
                                        MUL)
                nc.gpsimd.tensor_tensor(vall[:, 0:NQL], s2[:, :],
                                        s1sq[:, :], SUB)
                gsq = big.tile([WS, QT], BF16, tag="gsq", name=f"gsq{l}")
                nc.gpsimd.tensor_tensor(gsq[:, :], g[:, :], g[:, :], MUL)
                ps_s1s = pwork([WAY, QT], "s1s")
                nc.tensor.matmul(ps_s1s[:, :], BIND5, g[:, :],
                                 start=True, stop=True)
                ps_s2s = pwork([WAY, QT], "s2s")
                nc.tensor.matmul(ps_s2s[:, :], BIND, gsq[:, :],
                                 start=True, stop=True)
                s1ssq = small.tile([WAY, QT], BF16, tag="s1ssq",
                                   name=f"s1ssq{l}")
                nc.scalar.activation(s1ssq[:, :], ps_s1s[:, :], SQUARE)
                v24s = small.tile([WAY, QT], F32, tag="v24s",
                                  name=f"v24s{l}")
                nc.vector.tensor_tensor(v24s[:, :], ps_s2s[:, :],
                                        s1ssq[:, :], SUB)
                ps_vt = pwork([WS, NQL], "vt")
                nc.tensor.matmul(ps_vt[:, 0:WAY], v24s[:, 0:WS], IF5,
                                 is_transpose=True, start=True, stop=False)
                nc.tensor.matmul(ps_vt[:, WAY:NQL], v24s[:, WS:QT], IF5,
                                 is_transpose=True, start=False, stop=True)
                nc.vector.tensor_copy(vall[:, NQL:2 * NQL], ps_vt[:, :])

            def st5(l, v):
                """ivs = rsqrt((v24+bias)/24)/T, exact per-partition eps."""
                vall, bias3 = v["vall"], v["bias3"]
                lnv = small.tile([WS, 20], F32, tag="lnv", name=f"lnv{l}")
                nc.scalar.activation(lnv[:, 0:NQL], vall[:, 0:NQL], LN,
                                     bias=bias3[:, 0:1])
                nc.scalar.activation(lnv[:, NQL:NQL + WAY],
                                     vall[:, NQL:NQL + WAY], LN,
                                     bias=bias3[:, 1:2])
                nc.scalar.activation(lnv[:, NQL + WAY:2 * NQL],
                                     vall[:, NQL + WAY:2 * NQL], LN,
                                     bias=bias3[:, 2:3])
                ivs16 = small.tile([WS, 20], BF16, tag="ivs16",
                                   name=f"ivs16{l}")
                bexp = 0.5 * float(np.log(SS - 1.0)) - float(np.log(T_ATTN))
                nc.scalar.activation(ivs16[:, :], lnv[:, :], EXP,
                                     scale=-0.5, bias=bexp)
                v["ivs16"] = ivs16

            def st6(l, v):
                """t-softmax: e, den, f, aq, aqT."""
                y, ivs16 = v["y"], v["ivs16"]
                z = big.tile([WS, QT], BF16, tag="z", name=f"z{l}")
                nc.gpsimd.tensor_tensor(
                    z[:, :].rearrange("p (g t) -> p g t", t=SS),
                    y[:, :].rearrange("p (g t) -> p g t", t=SS),
                    ivs16[:, 0:NQL].unsqueeze(2).to_broadcast(
                        [WS, NQL, SS]), MUL)
                e = big.tile([WS, QT], BF16, tag="e", name=f"e{l}")
                nc.scalar.activation(e[:, :], z[:, :], EXP)
                v["e"] = e
                den = small.tile([WS, NQL], F32, tag="den", name=f"den{l}")
                nc.vector.tensor_reduce(
                    den[:, :], e[:, :].rearrange("p (g t) -> p g t", t=SS),
                    AX, ADD)
                rden = small.tile([WS, NQL], F32, tag="rden",
                                  name=f"rden{l}")
                nc.vector.reciprocal(rden[:, :], den[:, :])
                f = big.tile([WS, QT], BF16, tag="f", name=f"f{l}")
                nc.gpsimd.tensor_tensor(
                    f[:, :].rearrange("p (g t) -> p g t", t=SS),
                    e[:, :].rearrange("p (g t) -> p g t", t=SS),
                    rden[:, :].unsqueeze(2).to_broadcast([WS, NQL, SS]), MUL)
                v["f"] = f
                ps_aqT = pwork([WS, NQL], "aqT")
                nc.tensor.matmul(ps_aqT[:, 0:WAY], f[:, 0:WS], BIND,
                                 start=True, stop=False)
                nc.tensor.matmul(ps_aqT[:, WAY:NQL], f[:, WS:QT], BIND,
                                 start=False, stop=True)
                aqT16 = small.tile([WS, NQL], BF16, tag="aqT16",
                                   name=f"aqT16{l}")
                nc.vector.tensor_copy(aqT16[:, :], ps_aqT[:, :])
                v["aqT16"] = aqT16

            def st7(l, v):
                """s-softmax: es, dens, fs, As."""
                g, ivs16 = v["g"], v["ivs16"]
                ps_ivT = pwork([WAY, 256], "ivT", BF16)
                nc.tensor.matmul(ps_ivT[:, 0:WS], ivs16[:, NQL:NQL + WAY],
                                 I125, is_transpose=True, start=True,
                                 stop=False)
                nc.tensor.matmul(ps_ivT[:, 128:128 + WS],
                                 ivs16[:, NQL + WAY:2 * NQL], I125,
                                 is_transpose=True, start=False, stop=True)
                ivb16 = small.tile([WAY, 256], BF16, tag="ivb16",
                                   name=f"ivb16{l}")
                nc.vector.tensor_copy(ivb16[:, 0:WS], ps_ivT[:, 0:WS])
                nc.vector.tensor_copy(ivb16[:, 128:128 + WS],
                                      ps_ivT[:, 128:128 + WS])
                ps_ivb = pwork([WS, QT], "ivb")
                nc.tensor.matmul(ps_ivb[:, 0:WS], BINDT, ivb16[:, 0:WS],
                                 start=True, stop=False)
                nc.tensor.matmul(ps_ivb[:, WS:QT], BINDT,
                                 ivb16[:, 128:128 + WS],
                                 start=False, stop=True)
                zb = big.tile([WS, QT], BF16, tag="zb", name=f"zb{l}")
                nc.vector.tensor_tensor(zb[:, :], g[:, :], ps_ivb[:, :],
                                        MUL)
                es = big.tile([WS, QT], BF16, tag="es", name=f"es{l}")
                nc.scalar.activation(es[:, :], zb[:, :], EXP)
                ps_dens = pwork([WAY, QT], "dens")
                nc.tensor.matmul(ps_dens[:, :], BIND, es[:, :],
                                 start=True, stop=True)
                rdens16 = small.tile([WAY, QT], BF16, tag="rdens16",
                                     name=f"rdens{l}")
                with nc.allow_low_precision(reason="softmax denom bf16"):
                    nc.vector.reciprocal(rdens16[:, :], ps_dens[:, :])
                ps_rdb = pwork([WS, QT], "rdb")
                nc.tensor.matmul(ps_rdb[:, :], BINDT, rdens16[:, :],
                                 start=True, stop=True)
                fs = big.tile([WS, QT], BF16, tag="fs", name=f"fs{l}")
                nc.vector.tensor_tensor(fs[:, :], es[:, :], ps_rdb[:, :],
                                        MUL)
                As16 = small.tile([WS, NQL], BF16, tag="As16",
                                  name=f"As16{l}")
                with nc.allow_low_precision(reason="attn sums bf16"):
                    nc.vector.tensor_reduce(
                        As16[:, :],
                        fs[:, :].rearrange("p (g t) -> p g t", t=SS),
                        AX, ADD)
                v["As16"] = As16

            def st8(l, v):
                """pooled dot + |s_pool|^2 + |q_pool|^2 accumulation."""
                c2, As16, aqT16 = v["c2"], v["As16"], v["aqT16"]
                X = big.tile([WS, QT], BF16, tag="X", name=f"X{l}")
                nc.gpsimd.tensor_tensor(
                    X[:, :].rearrange("p (g t) -> p g t", t=SS),
                    c2[:, :].rearrange("p (g t) -> p g t", t=SS),
                    As16[:, :].unsqueeze(2).to_broadcast([WS, NQL, SS]),
                    MUL)
                ps_dot = pwork([WAY, QT], "dot")
                nc.tensor.matmul(ps_dot[:, :], BIND, X[:, :],
                                 start=True, stop=True)
                ps_aq = pwork([WAY, QT], "aq")
                nc.tensor.matmul(ps_aq[:, :], BIND, v["f"][:, :],
                                 start=True, stop=True)
                aq16 = small.tile([WAY, QT], BF16, tag="aq16",
                                  name=f"aq16{l}")
                nc.scalar.activation(aq16[:, :], ps_aq[:, :], COPY)
                P2 = small.tile([WAY, QT], F32, tag="P2", name=f"P2_{l}")
                nc.vector.tensor_tensor(P2[:, :], ps_dot[:, :],
                                        aq16[:, :], MUL)
                nc.vector.tensor_reduce(
                    dotacc[:, l * NQL:(l + 1) * NQL],
                    P2[:, :].rearrange("p (g t) -> p g t", t=SS), AX, ADD)
                # |s_pool|^2: way-blockdiag quadratic form via WMASK
                gsm = big.tile([WS, WS], BF16, tag="gsm", name=f"gsm{l}")
                nc.gpsimd.tensor_tensor(gsm[:, :], v["gs16"][:, :], WMASK,
                                        MUL)
                ps_py = pwork([WS, NQL], "py")
                nc.tensor.matmul(ps_py[:, :], gsm[:, :], As16[:, :],
                                 start=True, stop=True)
                zz16 = small.tile([WS, NQL], BF16, tag="zz16",
                                  name=f"zz16{l}")
                nc.vector.tensor_tensor(zz16[:, :], As16[:, :],
                                        ps_py[:, :], MUL)
                nc.tensor.matmul(acc[:, 0:NQL], BIND, zz16[:, :],
                                 start=(l == 0), stop=False)
                # |q_pool|^2: per-query blockdiag quadratic forms
                gq16 = v["gq16"]
                gqm = big.tile([WS, QT], BF16, tag="gqm", name=f"gqm{l}")
                nc.gpsimd.tensor_tensor(gqm[:, 0:WS], gq16[:, 0:WS],
                                        WMASK, MUL)
                nc.gpsimd.tensor_tensor(gqm[:, WS:QT], gq16[:, WS:QT],
                                        WMASK, MUL)
                ps_pz = pwork([WS, NQL], "pz")
                nc.tensor.matmul(ps_pz[:, 0:WAY], gqm[:, 0:WS],
                                 aqT16[:, 0:WAY], start=True, stop=False)
                nc.tensor.matmul(ps_pz[:, WAY:NQL], gqm[:, WS:QT],
                                 aqT16[:, WAY:NQL], start=False, stop=True)
                zq16 = small.tile([WS, NQL], BF16, tag="zq16",
                                  name=f"zq16{l}")
                nc.vector.tensor_tensor(zq16[:, :], aqT16[:, :],
                                        ps_pz[:, :], MUL)
                last = (l == NLV - 1)
                nc.tensor.matmul(acc[:, NQL:NQL + WAY], zq16[:, 0:WAY],
                                 BIND, start=False, stop=False)
                nc.tensor.matmul(acc[:, NQL + WAY:2 * NQL],
                                 zq16[:, WAY:NQL], BIND,
                                 start=False, stop=last)

            stages = [st0, st1, st2, st3, st4, st5, st6, st7, st8]
            for s_fn in stages:
                for l in range(NLV):
                    s_fn(l, V[l])

            # ---- final cosine ----
            dotsum = small.tile([WAY, NQL], F32, name="dotsum")
            nc.vector.tensor_reduce(
                dotsum[:, :],
                dotacc[:, :].rearrange("p (g t) -> p t g", t=NQL), AX, ADD)
            accs = small.tile([WAY, 2 * NQL], F32, name="accs")
            nc.vector.tensor_copy(accs[:, :], acc[:, :])
            den2 = small.tile([WAY, NQL], F32, name="den2")
            nc.vector.tensor_tensor(den2[:, :], accs[:, 0:NQL],
                                    accs[:, NQL:2 * NQL], MUL)
            invd = small.tile([WAY, NQL], F32, name="invd")
            nc.scalar.activation(invd[:, :], den2[:, :], LN)
            nc.scalar.activation(invd[:, :], invd[:, :], EXP, scale=-0.5,
                                 bias=float(np.log(5.0)))
            sim = small.tile([WAY, NQL], F32, name="sim")
            nc.vector.tensor_tensor(sim[:, :], dotsum[:, :], invd[:, :],
                                    MUL)
            nc.sync.dma_start(out=out_d[:, :], in_=sim[:, :])
    return nc


def _constants():
    import jax.numpy as jnp
    i125 = np.eye(WS, dtype=np.float32)
    bind = np.zeros((WS, WAY), dtype=np.float32)
    for w in range(WAY):
        bind[w * SS:(w + 1) * SS, w] = 1.0
    wmask = np.kron(np.eye(WAY, dtype=np.float32),
                    np.ones((SS, SS), dtype=np.float32))
    cb = np.zeros((P, NCB), dtype=np.float32)
    cb[0:WS, C_I:C_I + WS] = i125
    cb[0:WS, C_BI:C_BI + WAY] = bind
    cb[0:WS, C_B5:C_B5 + WAY] = bind * 0.2
    cb[0:WAY, C_BIT:C_BIT + WS] = bind.T
    cb[0:WS, C_WM:C_WM + WS] = wmask
    cb[0:1, C_OR:C_OR + P] = 1.0
    cb[0:P, C_OC:C_OC + 1] = 1.0
    cb[0:1, C_DV:C_DV + 4] = np.array(HYPER, dtype=np.float32)
    cf = np.zeros((P, NCF), dtype=np.float32)
    cf[0:WAY, 0:WAY] = np.eye(WAY, dtype=np.float32)
    return {
        "cb": np.asarray(jnp.asarray(cb, dtype=jnp.bfloat16)),
        "cf": cf,
    }


def _stage(spt: np.ndarray, qry: np.ndarray):
    """Host staging: pad, partition-major chunk layout (+ones col), bf16."""
    import jax.numpy as jnp
    NCH = C // P
    s = np.asarray(spt, dtype=np.float32).reshape(WAY, C, SS)
    sT = s.transpose(1, 0, 2).reshape(C, WS)          # [C, 125]
    s_pm = sT.reshape(NCH, P, WS).transpose(1, 0, 2).reshape(P, NCH * WS)
    s16 = np.asarray(jnp.asarray(s_pm, dtype=jnp.bfloat16))

    q = np.asarray(qry, dtype=np.float32).reshape(NQ, C, SS)
    qpad = np.zeros((NCORES * NQL, C, SS), dtype=np.float32)
    qpad[:NQ] = q
    qs16 = []
    for core in range(NCORES):
        qc = qpad[core * NQL:(core + 1) * NQL]        # [10, C, 25]
        qT = qc.transpose(1, 0, 2).reshape(C, QT)     # [C, 250]
        q_pm = qT.reshape(NCH, P, QT).transpose(1, 0, 2).reshape(P,
                                                                 NCH * QT)
        qs16.append(np.asarray(jnp.asarray(q_pm, dtype=jnp.bfloat16)))

    koff = np.cumsum([0] + LCH)
    s_lv = [np.ascontiguousarray(s16[:, koff[l] * SW:koff[l + 1] * SW])
            for l in range(NLV)]
    q_lv = [[np.ascontiguousarray(qc[:, koff[l] * QT:koff[l + 1] * QT])
             for l in range(NLV)] for qc in qs16]
    return s_lv, q_lv


def kernel(spt: np.ndarray, qry: np.ndarray) -> np.ndarray:
    if "nc" not in _CACHE:
        _CACHE["nc"] = _build_nc()
        _CACHE["consts"] = _constants()
    nc = _CACHE["nc"]
    consts = _CACHE["consts"]

    s_lv, q_lv = _stage(spt, qry)
    in_maps = []
    for core in range(NCORES):
        m = {f"s{l}": s_lv[l] for l in range(NLV)}
        m.update({f"q{l}": q_lv[core][l] for l in range(NLV)})
        m.update(consts)
        in_maps.append(m)

    res = run_bass_kernel_spmd(nc, in_maps, list(range(NCORES)))
    out = np.concatenate(
        [res.results[i]["out"].reshape(WAY, NQL).T for i in range(NCORES)],
        axis=0)
    return np.ascontiguousarray(out[:NQ])
